# revision 14
# baseline (speedup 1.0000x reference)
"""Trainium2 Bass kernel for ConvPool (3x3 VALID conv + bias + relu + 2x2 maxpool).

Full-input contract: kernel(x, weight, bias) -> (32, 64, 3969) float32.
Data-parallel over batch across 8 NeuronCores (4 images per core).

Per-core algorithm (v2):
  - All 4 images are convolved CONCURRENTLY in the four 64x64 quadrant
    blocks of the PE array (tile_position row/col tiling): img0 (rows 0-47,
    cols 0-63), img1 (64-111, 64-127), img2 (64-111, 0-63), img3 (0-47,
    64-127).  3 PSUM-accumulating matmuls per conv-row-tile (one per
    vertical tap n), K=48 (c x 3 horizontal taps m), N=512.
  - x is read from HBM ONCE per image (fp32->bf16 cast in the SWDGE DMA);
    the two m-shifted partition copies are built with on-chip SBUF->SBUF
    DMAs.  HBM read traffic: 4 MB/core instead of 12 MB.
  - Loads are chunked (4 chunks of 32 x-rows) so matmuls start early.
    Partition bases of the two x tiles are staggered (m0 group at
    partitions 0-15 in tile A, 16-31 in tile B) to spread the load DMAs
    over all 16 SDMA engines.
  - Post-PE: DVE h-max reads PSUM directly (max of column pairs, fp32 in /
    bf16 out) -- this halves the data volume at the first touch; DVE v-max
    over row pairs runs in bf16 2x mode; ScalarE applies relu(x + bias)
    (exact: max-pool commutes with the monotone x -> relu(x+b)) writing
    fp32 into the output stage; one batched output DMA per 4 tile-pairs.
  - Conv is padded to 128 rows (2 junk rows at the bottom, fed from a
    memset x pad) so every loop iteration is uniform; the junk pooled row
    lands in ostage padding that the output DMA never reads.
"""

import numpy as np

import concourse.bass as bass
import concourse.bacc as bacc
import concourse.mybir as mybir
import concourse.tile as tile
from concourse.bass_utils import run_bass_kernel_spmd

N_CORES = 8
B, C, H, W = 32, 16, 128, 128
FD, OUT, POOL = 3, 64, 2
BPC = B // N_CORES            # images per core
HC = H - FD + 1               # conv output height/width = 126
HP = HC // POOL               # pooled height/width = 63
NPIX = HP * HP                # 3969
HW = H * W                    # 16384
XPAD = 256                    # junk tail elems so conv rows 126/127 exist
OPAD = 4032                   # 32 row-tiles * 126 (63 junk cols at the end)
NCHUNK = 4                    # x load chunks (32 x-rows each)
CHW = HW // NCHUNK            # 4096 elems per chunk per partition

f32 = mybir.dt.float32
bf16 = mybir.dt.bfloat16
MAX = mybir.AluOpType.max

_cache: dict = {}

# quadrant assignment: img -> (xtile, partition base pb, psum col base cb,
#                              psum bank-pair bk)
#   img0: xA, rows 0-47,   cols 0-63    img1: xA, rows 64-111, cols 64-127
#   img2: xB, rows 64-111, cols 0-63    img3: xB, rows 0-47,   cols 64-127
# xA m-group partition order: m0@+0, m1@+16, m2@+32 (per 64-half)
# xB m-group partition order: m1@+0, m0@+16, m2@+32 (per 64-half)
#   (staggered so the HBM load DMAs of xA and xB hit disjoint SDMA engines)
A_M_BASE = {0: 0, 1: 16, 2: 32}
B_M_BASE = {0: 16, 1: 0, 2: 32}


def _build(loop_reps: int | None = None, mode: str = "full"):
    """Build the per-core program.  loop_reps wraps the whole body in a
    hardware For_i loop (benchmarking only: device time dominates wall).
    mode: 'full' | 'nopost' (skip DVE/ACT/out) | 'dmaonly' (x loads only)."""
    import contextlib

    nc = bacc.Bacc("TRN2", target_bir_lowering=False, debug=False)
    x_d = nc.dram_tensor("x", [BPC, C, H, W], f32, kind="ExternalInput").ap()
    w_d = nc.dram_tensor("weight", [C * FD * FD, OUT], f32,
                         kind="ExternalInput").ap()
    b_d = nc.dram_tensor("bias", [OUT], f32, kind="ExternalInput").ap()
    y_d = nc.dram_tensor("y", [BPC, OUT, NPIX], f32, kind="ExternalOutput").ap()

    with tile.TileContext(nc) as tc:
        with (
            tc.tile_pool(name="const", bufs=1) as const,
            tc.tile_pool(name="xrep", bufs=1) as xpool,
            tc.tile_pool(name="psum", bufs=2, space="PSUM") as psum,
            tc.tile_pool(name="hbuf", bufs=2) as hpool,
            tc.tile_pool(name="vbuf", bufs=2) as vpool,
            tc.tile_pool(name="ostage", bufs=1) as opool,
        ):
            # Weights at partition (half*64 + m_base + c), free (n*64 + o);
            # one tile per x-tile m-group order.
            w_src = w_d.rearrange("(c n m) o -> m c n o", c=C, n=FD, m=FD)
            w_sb = {}
            for key, m_base in (("A", A_M_BASE), ("B", B_M_BASE)):
                w_sb[key] = const.tile([128, FD * OUT], bf16,
                                       name=f"w{key}", tag=f"w{key}")
                for half in range(2):
                    for m in range(FD):
                        p0 = half * 64 + m_base[m]
                        dst = w_sb[key][p0:p0 + C, :].rearrange(
                            "p (n o) -> p n o", o=OUT)
                        nc.gpsimd.dma_start(dst, w_src[m])

            # Bias: per-partition scalar, duplicated for both halves.
            bias_sb = const.tile([128, 1], f32)
            b_src = b_d.rearrange("(o u) -> o u", u=1)
            nc.sync.dma_start(bias_sb[0:OUT, :], b_src)
            nc.sync.dma_start(bias_sb[OUT:128, :], b_src)

            loop_cm = (tc.For_i(0, loop_reps, 1) if loop_reps
                       else contextlib.nullcontext())
            with loop_cm:
                _body(nc, tc, x_d, y_d, w_sb, bias_sb,
                      xpool, psum, hpool, vpool, opool, mode)

    nc.compile()
    return nc


def _body(nc, tc, x_d, y_d, w_sb, bias_sb,
          xpool, psum, hpool, vpool, opool, mode="full"):
    xt = {"A": xpool.tile([128, HW + XPAD], bf16, name="xA", tag="xA"),
          "B": xpool.tile([128, HW + XPAD], bf16, name="xB", tag="xB")}

    # Junk tail: conv rows 126/127 stream from here; their pooled row lands
    # in ostage padding that the output DMA never reads.
    for key in ("A", "B"):
        nc.vector.memset(xt[key][:, HW:HW + XPAD], 0.0)

    # ---- x load: ONE wide SWDGE DMA (fp32->bf16 cast) into a staging
    # tile, partition = 32 + b*16 + c.  The offset-32 placement spans both
    # even and odd SDMA engine sets (16 engines).  SWDGE DMAs do not
    # pipeline, so one wide DMA beats many narrow ones (measured: 4 narrow
    # per-image loads = 44.5 us).
    stg = xpool.tile([128, HW], bf16, name="stg", tag="stg")
    x_src = x_d.rearrange("b c h w -> (b c) (h w)")
    nc.gpsimd.dma_start(stg[32:96, :], x_src)

    # ---- m-replication: HWDGE SBUF->SBUF copies from staging into the
    # x tiles, alternated across the two HWDGE rings (sync=SP, scalar=ACT).
    # Copy m holds x shifted left by m; the last m elems feed only conv
    # columns 126/127, which pooling never reads.
    # img -> (xtile key, partition half): batch b lives where its matmul
    # quadrant streams from (see `quad` below).
    LOADS = ((0, "A", 0), (1, "A", 1), (2, "B", 1), (3, "B", 0))
    ring = 0
    for b_idx, key, half in LOADS:
        m_base = A_M_BASE if key == "A" else B_M_BASE
        src = stg[32 + b_idx * C:32 + (b_idx + 1) * C, :]
        for mm in range(FD):
            pd = half * 64 + m_base[mm]
            eng = nc.sync if ring % 2 == 0 else nc.scalar
            ring += 1
            eng.dma_start(xt[key][pd:pd + C, 0:HW - mm], src[:, mm:HW])
    if mode == "dmaonly":
        return

    ost = opool.tile([128, 2 * OPAD], f32)

    # ---- compute: 16 double-row-tiles of 2 sub-tiles x 2 bank-pairs ----
    # img -> (xtile key, partition base, psum col base, bank-pair)
    quad = ((0, "A", 0, 0, 0), (1, "A", 64, 64, 0),
            (2, "B", 64, 0, 1), (3, "B", 0, 64, 1))
    for tt in range(16):
        pst = psum.tile([128, 2048], f32)
        for n in range(FD):
            for _img, key, pb, cb, bk in quad:
                for s in range(2):
                    t = 2 * tt + s
                    off = (4 * t + n) * W
                    lhsT = w_sb[key][pb:pb + FD * C, n * OUT:(n + 1) * OUT]
                    rhs = xt[key][pb:pb + FD * C, off:off + 512]
                    out = pst[cb:cb + OUT, s * 1024 + bk * 512:
                              s * 1024 + bk * 512 + 512]
                    nc.tensor.matmul(out, lhsT, rhs,
                                     start=(n == 0), stop=(n == FD - 1))
        if mode == "nopost":
            continue

        # ScalarE evacuates the ODD conv columns with relu+bias fused
        # (bf16 out).  DVE then computes (even + bias) max relu_odd, which
        # equals relu(max(even, odd) + bias) exactly -- only ONE operand
        # reads PSUM (hardware limit).
        ps_v = pst.rearrange("p (g j2 two) -> p g j2 two", two=2, j2=64)
        ob = hpool.tile([128, 1024], bf16, name="ob", tag="ob")
        ob_v = ob.rearrange("p (g j) -> p g j", j=64)
        nc.scalar.activation(ob_v, ps_v[:, :, :, 1],
                             mybir.ActivationFunctionType.Relu,
                             bias=bias_sb[:, 0:1], scale=1.0)

        hb = hpool.tile([128, 1024], bf16, name="hb", tag="hb")
        hb_v = hb.rearrange("p (g j) -> p g j", j=64)
        nc.vector.scalar_tensor_tensor(
            out=hb_v, in0=ps_v[:, :, :, 0], scalar=bias_sb[:, 0:1],
            in1=ob_v, op0=mybir.AluOpType.add, op1=MAX)

        # v-max over row pairs: the result IS the final pooled output.
        hb_p = hb.rearrange("p (s b rp two j) -> p s b rp two j",
                            s=2, b=2, rp=2, two=2, j=64)
        ost_v = (ost.rearrange("p (b q) -> p b q", b=2)
                 [:, :, tt * 252:tt * 252 + 252]
                 .rearrange("p b (s rp j) -> p s b rp j", s=2, rp=2))
        if tt % 8 == 7:
            # DVE writes fp32-packed ostage directly (1x mode) -- keeps
            # ScalarE/DVE balanced.
            nc.vector.tensor_tensor(out=ost_v,
                                    in0=hb_p[:, :, :, :, 0, 0:HP],
                                    in1=hb_p[:, :, :, :, 1, 0:HP], op=MAX)
        else:
            # bf16 2x-mode v-max into a padded staging tile, then ScalarE
            # packs+casts into the fp32 output stage.
            vb = vpool.tile([128, 512], bf16)
            vb_v = vb.rearrange("p (s b rp j) -> p s b rp j",
                                s=2, b=2, rp=2)
            nc.vector.tensor_tensor(out=vb_v, in0=hb_p[:, :, :, :, 0, :],
                                    in1=hb_p[:, :, :, :, 1, :], op=MAX)
            nc.scalar.activation(
                ost_v, vb_v[:, :, :, :, 0:HP],
                mybir.ActivationFunctionType.Copy, bias=0.0, scale=1.0)

        # Batched output DMA every 4 double-tiles (4KB per partition line).
        if mode == "full" and tt % 4 == 3:
            q0 = (tt - 3) * 252
            q1 = min(q0 + 1008, NPIX)
            for pair in range(2):
                dst = y_d[2 * pair:2 * pair + 2].rearrange(
                    "b o q -> (b o) q")[:, q0:q1]
                nc.sync.dma_start(dst, ost[:, pair * OPAD + q0:
                                           pair * OPAD + q1])


def _get_nc():
    if "nc" not in _cache:
        _cache["nc"] = _build()
    return _cache["nc"]


def kernel(x: np.ndarray, weight: np.ndarray, bias: np.ndarray) -> np.ndarray:
    nc = _get_nc()
    x = np.ascontiguousarray(x, dtype=np.float32)
    weight = np.ascontiguousarray(weight, dtype=np.float32)
    bias = np.ascontiguousarray(bias, dtype=np.float32)
    xs = x.reshape(N_CORES, BPC, C, H, W)
    in_maps = [{"x": xs[i], "weight": weight, "bias": bias}
               for i in range(N_CORES)]
    res = run_bass_kernel_spmd(nc, in_maps, list(range(N_CORES)))
    return np.concatenate([res.results[i]["y"] for i in range(N_CORES)], axis=0)
